# revision 59
# baseline (speedup 1.0000x reference)
"""Sparse-attention head kernel for Trainium2, data-parallel over batch on 8 cores.

v6 design (per core, one batch):
  - DMA: all large input transfers ride the sync HW queue as few, large
    dma_starts with 4-8KB contiguous runs per partition (host lays x out
    half-range-major).  Consts ride the gpsimd queue.  The fp8 copy of the
    gathered-q tail rows is NOT transferred: keep[1024:2048] == t 3072..4095,
    so q-chunks 2,3 reuse xt8 groups 6,7.
  - q and k are projected W-stationary with fp8 DoubleRow into transposed
    fp8 layouts (qgT [64, 2*2048], kT [64, 1024/group]); v is projected
    x-stationary bf16 per t-block, with the bias-add on the Pool engine.
  - S runs fp8 DoubleRow (zero-padded pair slot): 2x column rate.
  - exp on ACT with the 1/sqrt(C) scale folded; boundary-block masks
    multiply e on DVE.
  - PV accumulates out^T into two pinned PSUM tiles [65, 1024] (row 64 =
    softmax denominator via the ones column); first writer per PSUM column
    region uses start=True (no explicit zeroing pass).
  - epilogue: j0..3 staged during the sweep; j4..7 and j8..15 batched
    (single PSUM->SBUF copy, packed PE transposes, batched reciprocal,
    scales split across ACT and DVE).
"""

import math
import os

if "JAX_PLATFORMS" not in os.environ:
    os.environ["JAX_PLATFORMS"] = "axon,cpu"

import numpy as np
import ml_dtypes

B, T, C = 8, 4096, 1024
HS = 64
KQ = T // 2
NCORES = 8
SCALE = float(C) ** -0.5
BF16 = ml_dtypes.bfloat16
FP8NP = ml_dtypes.float8_e4m3

_NT = T // 128   # 32
_NJ = KQ // 128  # 16
_NG = T // 512   # 8 k-projection groups
_NQ = KQ // 512  # 4 q-projection chunks (2,3 alias xt8 g6,g7)

# half-ranges of t, in DMA/layout order (tail first: prologue needs it)
_HR = [(3072, 3584), (3584, 4096), (0, 512), (512, 1024), (1024, 1536),
       (1536, 2048), (2048, 2560), (2560, 3072)]
_HRIDX = {lo // 512: i for i, (lo, hi) in enumerate(_HR)}


def _keep_indices(t):
    a = math.ceil(t / 4)
    keep = [t - 1 - x for x in range(a)]
    keep += [t - 1 - math.ceil(3 / a * (x - a) ** 2 + a) for x in range(a, math.ceil(t / 2))]
    return np.array(list(reversed(keep)), dtype=np.int64)


KEEP = _keep_indices(T)

_FULL, _BOUND, _DEAD = 0, 1, 2
_BLOCK_KIND = np.empty((_NT, _NJ), dtype=np.int64)
for _tb in range(_NT):
    for _j in range(_NJ):
        qlo, qhi = KEEP[_j * 128], KEEP[_j * 128 + 127]
        if 128 * _tb + 127 <= qlo:
            _BLOCK_KIND[_tb, _j] = _FULL
        elif 128 * _tb > qhi:
            _BLOCK_KIND[_tb, _j] = _DEAD
        else:
            _BLOCK_KIND[_tb, _j] = _BOUND

_MASK_IDX = {}
for _tb in range(_NT):
    for _j in range(8):
        if _BLOCK_KIND[_tb, _j] == _BOUND:
            _MASK_IDX[(_tb, _j)] = len(_MASK_IDX)
_NMASK = len(_MASK_IDX)
_TRIL_IDX = _NMASK

_JLO = np.empty(_NT, dtype=np.int64)
for _tb in range(_NT):
    _JLO[_tb] = _NJ
    for _j in range(_NJ):
        if _BLOCK_KIND[_tb, _j] != _DEAD:
            _JLO[_tb] = _j
            break

# last alive t-block per q-block j (column j's accumulation completes there)
_JLAST = np.empty(_NJ, dtype=np.int64)
for _j in range(_NJ):
    _JLAST[_j] = (int(KEEP[_j * 128 + 127]) // 128)


def _host_masks():
    m = np.zeros((128, (_NMASK + 1) * 128), dtype=np.float32)
    for (tb, j), idx in _MASK_IDX.items():
        tvals = 128 * tb + np.arange(128)[:, None]
        kvals = KEEP[j * 128:(j + 1) * 128][None, :]
        m[:, idx * 128:(idx + 1) * 128] = (tvals <= kvals).astype(np.float32)
    r = np.arange(128)[:, None]
    c = np.arange(128)[None, :]
    m[:, _TRIL_IDX * 128:(_TRIL_IDX + 1) * 128] = (r <= c).astype(np.float32)
    return m.astype(BF16)


_prog_cache = {}
TRACE = False
TRACE_KW = {}
LAST_RESULTS = None


_KNOBS = {
    "WARMUP_N": 6,     # PE warm-up dummy matmuls
    "PV_KEEP": 6,      # pending PV depth
    "EP_BUFS": 8,      # e-tile pool depth
    "MASK_POOL": 0,    # mask multiplies on Pool instead of DVE
}


def _knob(name):
    return int(os.environ.get(f"K_{name}", _KNOBS[name]))


def _build_program():
    import concourse.bass as bass  # noqa: F401
    import concourse.mybir as mybir
    import concourse.tile as tile
    from concourse import bacc
    from concourse.masks import make_identity

    dt = mybir.dt
    f32, bf16, fp8 = dt.float32, dt.bfloat16, dt.float8e4
    Alu = mybir.AluOpType
    Act = mybir.ActivationFunctionType
    DR = mybir.MatmulPerfMode.DoubleRow

    nc = bacc.Bacc("TRN2", target_bir_lowering=False, debug=False,
                   enable_partition_id=False)

    # x bf16, half-range-major: [128, hr(8) x c(8) x 512]
    xt_d = nc.dram_tensor("xt", [128, 8 * 4096], bf16, kind="ExternalInput").ap()
    # fp8 c-pair k-outer: per group g (512 t): 4096 cols = [gp(4)][two(2)][512]
    xt8_d = nc.dram_tensor("xt8", [128, _NG * 4096], fp8, kind="ExternalInput").ap()
    # only q-chunks 0,1 (quadratic keep rows); chunks 2,3 alias xt8 g6,g7
    xq8_d = nc.dram_tensor("xq8", [128, 2 * 4096], fp8, kind="ExternalInput").ap()
    # consts packed into two byte blobs (one dma_start each: per-dma
    # sequencer cost on the issuing engine is ~1us)
    # blob1: wk8 (512B) | wq8 (512B) | bk f32 (4B, parts 0..63) | bq f32
    blob1_d = nc.dram_tensor("blob1", [128, 1032], dt.uint8,
                             kind="ExternalInput").ap()
    # blob2: masks ((_NMASK+1)*256 B) | wv (1024B) | bv (128B)
    _B2M = (_NMASK + 1) * 256
    blob2_d = nc.dram_tensor("blob2", [128, _B2M + 1152], dt.uint8,
                             kind="ExternalInput").ap()
    out_d = nc.dram_tensor("out", [KQ, HS], f32, kind="ExternalOutput").ap()

    with tile.TileContext(nc) as tc:
        with (
            tc.tile_pool(name="const", bufs=1) as constp,
            tc.tile_pool(name="xt", bufs=1) as xtp,
            tc.tile_pool(name="proj", bufs=1) as projp,
            tc.tile_pool(name="psS", bufs=2, space="PSUM") as psS,
            tc.tile_pool(name="psO", bufs=1, space="PSUM") as psO,
            tc.tile_pool(name="work", bufs=2) as workp,
            tc.tile_pool(name="ework", bufs=_knob("EP_BUFS")) as ep,
        ):
            # ---- consts: two blob dma_starts on the scalar HW queue (fast
            # start; per-dma sequencer cost is ~1us, so batch them).  xt8
            # g7 rides between them so it transfers in parallel with g6 on
            # the sync queue. ----
            blob1 = constp.tile([128, 1032], dt.uint8)
            nc.scalar.dma_start(out=blob1, in_=blob1_d)
            blob2 = constp.tile([128, _B2M + 1152], dt.uint8)
            nc.scalar.dma_start(out=blob2, in_=blob2_d)
            wk8_sb = blob1[:, 0:512].bitcast(fp8)
            wq8_sb = blob1[:, 512:1024].bitcast(fp8)
            bk_sb = blob1[0:64, 1024:1028].bitcast(f32)
            bq_sb = blob1[0:64, 1028:1032].bitcast(f32)
            mask_big = blob2[:, 0:_B2M].bitcast(bf16)
            wv_sb = blob2[:, _B2M:_B2M + 1024].bitcast(bf16)
            bv_bc = blob2[:, _B2M + 1024:_B2M + 1152].bitcast(bf16)
            # PE warm-up: dummy matmuls on a zeroed tile while input DMAs
            # are in flight, so the PE p-state is ramped when real work
            # lands (the dummy memset is first in the gpsimd stream).
            dummy = constp.tile([128, 512], bf16)
            nc.gpsimd.memset(dummy, 0.0)
            for _w in range(_knob("WARMUP_N")):
                ps_w = psS.tile([128, 512], f32, name="ps", tag="ps")
                nc.tensor.matmul(ps_w, lhsT=dummy[:, 0:128], rhs=dummy,
                                 start=True, stop=True)

            # ---- persistent tensors ----
            xt_h = [xtp.tile([128, 4096], bf16, name=f"xth_{i}",
                             tag=f"xth_{i}") for i in range(8)]
            xt8_g = [xtp.tile([128, 4096], fp8, name=f"xt8_{g}",
                              tag=f"xt8_{g}") for g in range(_NG)]
            xq8_g = [xtp.tile([128, 4096], fp8, name=f"xq8_{g}",
                              tag=f"xq8_{g}") for g in range(2)]
            # S-critical zero halves go on DVE (idle until ~6us; Pool is
            # busy with the other memsets); identity last (epilogue-only)
            kt_g = [projp.tile([64, 1024], fp8, name=f"kt_{g}",
                               tag=f"kt_{g}") for g in range(_NG)]
            qgt8 = projp.tile([64, 2 * KQ], fp8)
            nc.vector.memset(qgt8[:, KQ:2 * KQ].bitcast(dt.uint32), 0)
            nc.vector.memset(kt_g[6][:, 512:1024].bitcast(dt.uint32), 0)
            nc.vector.memset(kt_g[7][:, 512:1024].bitcast(dt.uint32), 0)
            for g in (0, 1, 2, 3, 4, 5):
                nc.gpsimd.memset(kt_g[g][:, 512:1024].bitcast(dt.uint32), 0)
            vext_sb = [projp.tile([128, HS + 1], bf16, name=f"vext_{tb}",
                                  tag=f"vext_{tb}") for tb in range(_NT)]
            for tb in range(_NT):
                nc.gpsimd.memset(vext_sb[tb][:, HS:HS + 1], 1.0)
            ident_f = constp.tile([128, 128], f32)
            make_identity(nc, ident_f)
            ps_o_lo = psO.tile([65, 1024], f32, name="po_lo", tag="po_lo")
            ps_o_hi = psO.tile([65, 1024], f32, name="po_hi", tag="po_hi")

            # ---- big input DMAs, all on the sync HW queue, in need-order ----
            def dma_xt8(g):
                nc.sync.dma_start(out=xt8_g[g],
                                  in_=xt8_d[:, g * 4096:(g + 1) * 4096])

            def dma_xth(i):
                nc.sync.dma_start(out=xt_h[i],
                                  in_=xt_d[:, i * 4096:(i + 1) * 4096])

            def dma_xq8(c):
                nc.sync.dma_start(out=xq8_g[c],
                                  in_=xq8_d[:, c * 4096:(c + 1) * 4096])

            # order feeds the ACT exp stream continuously: k-data first
            # (unlocks strip + hi-prepass S/exp with PV deferred), then the
            # quadratic-q and v data for the dense sweep, with the strips' v
            # ranges (t 3072..4095) arriving late since their PV is deferred.
            dma_xt8(6)
            dma_xt8(7)
            dma_xt8(0)
            dma_xt8(1)
            dma_xq8(0)
            dma_xq8(1)
            dma_xth(2)   # t 0..511    (v tb0..3)
            dma_xth(3)   # t 512..1023
            dma_xt8(2)
            dma_xth(4)   # t 1024..1535
            dma_xt8(3)
            dma_xth(5)
            dma_xt8(4)
            dma_xth(6)   # t 2048..2559
            dma_xt8(5)
            dma_xth(7)   # t 2560..3071 (v tb20..23)
            dma_xth(0)   # t 3072..3583 (v tb24..27, strips)
            dma_xth(1)   # t 3584..4095 (v tb28..31, strips)

            def xt_sl(c, lo, hi):
                hr = _HRIDX[lo // 512]
                o = c * 512 + (lo - _HR[hr][0])
                return xt_h[hr][:, o:o + (hi - lo)]

            # ---- W-stationary fp8-DR projections (k and q) ----
            def emit_kproj(g):
                ps_k = psS.tile([64, 512], f32, name="ps", tag="ps")
                for gp in range(4):
                    lhs = wk8_sb[:, gp * 128:(gp + 1) * 128].rearrange(
                        "p (two m) -> p two m", two=2)
                    rhs = xt8_g[g][:, gp * 1024:(gp + 1) * 1024].rearrange(
                        "p (two n) -> p two n", two=2)
                    nc.tensor.matmul(ps_k, lhsT=lhs, rhs=rhs,
                                     start=(gp == 0), stop=(gp == 3),
                                     perf_mode=DR)
                nc.vector.tensor_scalar(out=kt_g[g][:, 0:512], in0=ps_k,
                                        scalar1=bk_sb, scalar2=None,
                                        op0=Alu.add)

            def emit_qproj(c):
                src = xq8_g[c] if c < 2 else xt8_g[4 + c]
                ps_q = psS.tile([64, 512], f32, name="ps", tag="ps")
                for gp in range(4):
                    lhs = wq8_sb[:, gp * 128:(gp + 1) * 128].rearrange(
                        "p (two m) -> p two m", two=2)
                    rhs = src[:, gp * 1024:(gp + 1) * 1024].rearrange(
                        "p (two n) -> p two n", two=2)
                    nc.tensor.matmul(ps_q, lhsT=lhs, rhs=rhs,
                                     start=(gp == 0), stop=(gp == 3),
                                     perf_mode=DR)
                nc.vector.tensor_scalar(out=qgt8[:, c * 512:(c + 1) * 512],
                                        in0=ps_q, scalar1=bq_sb, scalar2=None,
                                        op0=Alu.add)

            # ---- v projection per t-block (x-stationary; the 8-matmul
            # accumulation burst pipelines at stream rate).  When a host
            # PSUM slice is provided (unused tail of an S tile), no psS pool
            # allocation happens — keeping the pool rotation at 2 allocs per
            # t-block avoids a full-exp-duration WAR bubble. ----
            def emit_vproj(tb, ps_slice=None):
                t0 = tb * 128
                ps_v = ps_slice
                skip = ps_slice is not None
                if ps_v is None:
                    ps_v = psS.tile([128, 64], f32, name="ps", tag="ps")
                for c in range(8):
                    nc.tensor.matmul(ps_v, lhsT=xt_sl(c, t0, t0 + 128),
                                     rhs=wv_sb[:, c * 64:(c + 1) * 64],
                                     start=(c == 0), stop=(c == 7),
                                     skip_group_check=skip)
                nc.vector.tensor_tensor(out=vext_sb[tb][:, 0:HS], in0=ps_v,
                                        in1=bv_bc, op=Alu.add)

            # ---- attention ----
            def s_matmul(ps_slice, tb, q0, q1):
                g, o = tb // 4, (tb % 4) * 128
                lhs = kt_g[g].rearrange("p (two m) -> p two m",
                                        two=2)[:, :, o:o + 128]
                rhs = qgt8.rearrange("p (two n) -> p two n", two=2)[:, :, q0:q1]
                nc.tensor.matmul(ps_slice, lhsT=lhs, rhs=rhs,
                                 start=True, stop=True, perf_mode=DR)

            pending_pv = []

            def flush_pv(keep=0):
                while len(pending_pv) > keep:
                    tb, e_sl, q0, qm, stop = pending_pv.pop(0)
                    p0 = q0
                    while p0 < qm:
                        pb = min((p0 // 512 + 1) * 512, qm)
                        tgt = ps_o_lo if p0 < 1024 else ps_o_hi
                        base = 0 if p0 < 1024 else 1024
                        # first writer zeroes its own columns (no init pass):
                        # tb0 writes all lo columns via att_lo(0) and all hi
                        # columns via the deferred hi-prepass entry.
                        nc.tensor.matmul(
                            tgt[:, p0 - base:pb - base], lhsT=vext_sb[tb],
                            rhs=e_sl[:, p0 - q0:pb - q0],
                            start=(tb == 0), stop=stop, skip_group_check=True)
                        p0 = pb

            def mask_block(e_sb, tb, j, o):
                midx = None
                if j < 8 and _BLOCK_KIND[tb, j] == _BOUND:
                    midx = _MASK_IDX[(tb, j)]
                elif j >= 8 and tb == j + 16:
                    midx = _TRIL_IDX
                if midx is not None:
                    eng = nc.gpsimd if _knob("MASK_POOL") else nc.vector
                    eng.tensor_tensor(
                        out=e_sb[:, o:o + 128], in0=e_sb[:, o:o + 128],
                        in1=mask_big[:, midx * 128:(midx + 1) * 128],
                        op=Alu.mult)

            def emit_att(tb, q0, q1, stop=False, fillers=()):
                first = True
                while q0 < q1:
                    qm = min((q0 // 1024 + 1) * 1024, q1)
                    ps_s = psS.tile([128, 1024], f32, name="ps", tag="ps")
                    for s0 in range(q0, qm, 512):
                        s1 = min(s0 + 512, qm)
                        s_matmul(ps_s[:, s0 - q0:s1 - q0], tb, s0, s1)
                    prev = list(pending_pv)
                    pending_pv.clear()
                    e_sb = ep.tile([128, 1024], bf16, name="e_sb")
                    w = qm - q0
                    nc.scalar.activation(e_sb[:, 0:w], ps_s[:, 0:w],
                                         Act.Exp, scale=SCALE)
                    for j in range(q0 // 128, qm // 128):
                        mask_block(e_sb, tb, j, j * 128 - q0)
                    pending_pv.extend(prev)
                    flush_pv(keep=_knob("PV_KEEP"))
                    pending_pv.append((tb, e_sb[:, 0:w], q0, qm, stop))
                    if first:
                        # off-critical-path work lands between the S chunks;
                        # fillers may claim unused 64-col tails of this
                        # chunk's PSUM tile (avoids extra pool allocations)
                        ctx = {"ps": ps_s, "next": 1024, "limit": w}
                        for f in fillers:
                            f(ctx)
                        first = False
                    q0 = qm

            def _ps64(ctx):
                # carve a [128, 64] slice off the current S tile's free tail
                if ctx is not None and ctx["next"] - 64 >= ctx["limit"]:
                    ctx["next"] -= 64
                    return ctx["ps"][:, ctx["next"]:ctx["next"] + 64]
                return None

            # ---- deferred S/exp producers (PV injected later) ----
            # strips (t-blocks 24..31) and the hi-prepass (tb 0..5 over
            # q 1024..2048) run S+exp early, holding e in dedicated tiles;
            # their PV entries are injected into pending_pv once the
            # corresponding vext data has arrived.
            estrip = [projp.tile([128, 1024], bf16, name=f"es_{i}",
                                 tag=f"es_{i}") for i in range(6)]
            eahi = [projp.tile([128, 1024], bf16, name=f"eh_{tb}",
                               tag=f"eh_{tb}") for tb in range(8)]
            strips_pv = {tb: [] for tb in range(24, 32)}
            ahi_pv = {}
            _strip_tile = [0]

            def emit_strip_pair(p):
                j0 = 8 + 2 * p
                items = []
                for tb in range(24, 26 + 2 * p):
                    c0 = max(j0 * 128, (tb - 16) * 128)
                    c1 = (j0 + 2) * 128
                    if c0 < c1:
                        items.append((tb, c0, c1))

                def flush_group(g):
                    if not g:
                        return
                    ps_s = psS.tile([128, 1024], f32, name="ps", tag="ps")
                    ofs = 0
                    placed = []
                    for (tb, c0, c1) in g:
                        s_matmul(ps_s[:, ofs:ofs + (c1 - c0)], tb, c0, c1)
                        placed.append((tb, c0, c1, ofs))
                        ofs += c1 - c0
                    e_sb = estrip[_strip_tile[0]]
                    _strip_tile[0] += 1
                    nc.scalar.activation(e_sb[:, 0:ofs], ps_s[:, 0:ofs],
                                         Act.Exp, scale=SCALE)
                    for (tb, c0, c1, o) in placed:
                        for j in range(c0 // 128, c1 // 128):
                            mask_block(e_sb, tb, j, o + j * 128 - c0)
                    for (tb, c0, c1, o) in placed:
                        strips_pv[tb].append(
                            (tb, e_sb[:, o:o + (c1 - c0)], c0, c1, False))

                g, used = [], 0
                for it in items:
                    wdt = it[2] - it[1]
                    if used + wdt > 1024:
                        flush_group(g)
                        g, used = [], 0
                    g.append(it)
                    used += wdt
                flush_group(g)

            def emit_att_hi(tb):
                # hi-prepass: S+exp for q 1024..2048 of t-block tb, deferred
                ps_s = psS.tile([128, 1024], f32, name="ps", tag="ps")
                s_matmul(ps_s[:, 0:512], tb, 1024, 1536)
                s_matmul(ps_s[:, 512:1024], tb, 1536, 2048)
                e_sb = eahi[tb]
                nc.scalar.activation(e_sb, ps_s, Act.Exp, scale=SCALE)
                ahi_pv[tb] = (tb, e_sb[:, 0:1024], 1024, 2048, False)

            # ---- epilogue ----
            out8_lo = workp.tile([128, 8 * HS], f32, name="out8lo", tag="o8lo")
            out8_hi = workp.tile([128, 8 * HS], f32, name="out8hi", tag="o8hi")

            def emit_epi_j(j):
                # staged single-j epilogue (used for early-finishing lo j's)
                ps_o = ps_o_lo if j < 8 else ps_o_hi
                base = 0 if j < 8 else 1024
                out8 = out8_lo if j < 8 else out8_hi
                jj = j % 8
                ot = workp.tile([65, 128], f32, name="ot", tag=f"ot{j % 2}")
                nc.vector.tensor_copy(
                    ot, ps_o[:, j * 128 - base:(j + 1) * 128 - base])
                ps_on = psS.tile([128, 65], f32, name="ps", tag="ps")
                nc.tensor.transpose(ps_on, ot, ident_f[0:65, 0:65])
                rec = workp.tile([128, 1], f32, name="rec", tag=f"rec{j % 2}")
                nc.vector.reciprocal(rec, ps_on[:, HS:HS + 1])
                nc.vector.tensor_scalar(
                    out=out8[:, jj * HS:(jj + 1) * HS], in0=ps_on[:, 0:HS],
                    scalar1=rec[:, :1], scalar2=None, op0=Alu.mult)

            def emit_epi_batch(jlist, on_act):
                # batched: one PSUM->SBUF copy, packed transposes, one
                # reciprocal, per-j scale on ACT or DVE.
                n = len(jlist)
                j0 = jlist[0]
                ps_o = ps_o_lo if j0 < 8 else ps_o_hi
                base = 0 if j0 < 8 else 1024
                out8 = out8_lo if j0 < 8 else out8_hi
                ot = workp.tile([65, n * 128], f32, name=f"otb{j0}",
                                tag=f"otb{j0 % 2}")
                nc.vector.tensor_copy(
                    ot, ps_o[:, j0 * 128 - base:(j0 + n) * 128 - base])
                pack = psS.tile([128, 1024], f32, name="ps", tag="ps")
                for i in range(n):
                    nc.tensor.matmul(
                        pack[:, i * 65:(i + 1) * 65],
                        lhsT=ot[:, i * 128:(i + 1) * 128],
                        rhs=ident_f[0:65, 0:65], is_transpose=True,
                        skip_group_check=True)
                rec = workp.tile([128, n], f32, name=f"recb{j0}",
                                 tag=f"recb{j0 % 2}")
                packv = pack[:, 0:n * 65].rearrange("p (i c) -> p i c", c=65)
                nc.vector.reciprocal(rec, packv[:, :, HS:HS + 1])
                for i, j in enumerate(jlist):
                    jj = j % 8
                    if on_act:
                        nc.scalar.activation(
                            out8[:, jj * HS:(jj + 1) * HS],
                            pack[:, i * 65:i * 65 + HS],
                            Act.Copy, scale=rec[:, i:i + 1])
                    else:
                        nc.vector.tensor_scalar(
                            out=out8[:, jj * HS:(jj + 1) * HS],
                            in0=pack[:, i * 65:i * 65 + HS],
                            scalar1=rec[:, i:i + 1], scalar2=None,
                            op0=Alu.mult)

            def emit_out_dma(which):
                if which == "lo":
                    out8, qbase, j0, nj = out8_lo, 0, 0, 8
                elif which == "q8_11":
                    out8, qbase, j0, nj = out8_hi, 1024, 0, 4
                else:
                    out8, qbase, j0, nj = out8_hi, 1536, 4, 4
                out_view = out_d[qbase:qbase + nj * 128, :].rearrange(
                    "(j p) d -> p j d", p=128)
                nc.sync.dma_start(
                    out=out_view,
                    in_=out8[:, j0 * HS:(j0 + nj) * HS].rearrange(
                        "p (j d) -> p j d", j=nj))

            # ---- schedule ----
            # prologue ordered by data-readiness: strips 0-1 need only
            # g6 + q-chunk2, so they start as soon as the first k group
            # lands; the ACT exp stream then never starves.
            emit_kproj(6)
            emit_qproj(2)
            emit_strip_pair(0)
            emit_strip_pair(1)
            emit_kproj(7)
            emit_qproj(3)
            emit_kproj(0)
            emit_att_hi(0)
            emit_att_hi(1)
            emit_strip_pair(2)
            emit_strip_pair(3)
            emit_att_hi(2)
            emit_att_hi(3)
            emit_kproj(1)
            for tb in range(4, 8):
                emit_att_hi(tb)
            emit_qproj(0)
            emit_qproj(1)
            emit_vproj(0)
            emit_vproj(1)
            # dense sweep; v-proj leads by 2 t-blocks, k groups 2..5 and the
            # strip PV injections placed just behind their DMA arrivals.
            kg_at = {7: 2, 11: 3, 15: 4, 18: 5}
            epi_q = [0, 1, 2, 3, 4]  # staged lo epilogues by _JLAST
            for tb in range(0, 24):
                fillers = []
                if tb + 2 < 24:
                    fillers.append(lambda c, t=tb + 2: emit_vproj(t, _ps64(c)))
                if tb == 16 or tb == 17:
                    for stb in (24 + 2 * (tb - 16), 25 + 2 * (tb - 16)):
                        fillers.append(lambda c, t=stb: emit_vproj(t, _ps64(c)))
                if tb == 18:
                    for stb in range(28, 32):
                        fillers.append(lambda c, t=stb: emit_vproj(t, _ps64(c)))
                if tb in kg_at:
                    fillers.append(lambda c, g=kg_at[tb]: emit_kproj(g))
                # staging margin: pending PV holds ~PV_KEEP+1 entries
                # (~(PV_KEEP+1)/2 t-blocks), so column j's last PV piece is
                # only guaranteed emitted this many t-blocks after _JLAST
                margin = _knob("PV_KEEP") // 2 + 2
                while epi_q and _JLAST[epi_q[0]] <= tb - margin:
                    fillers.append(lambda c, j=epi_q.pop(0): emit_epi_j(j))
                if tb <= 7:
                    pending_pv.append(ahi_pv[tb])
                    emit_att(tb, int(_JLO[tb]) * 128, 1024, fillers=fillers)
                else:
                    emit_att(tb, int(_JLO[tb]) * 128, 2048, stop=(tb == 23),
                             fillers=fillers)
                if tb == 17:
                    pending_pv.extend(strips_pv[24] + strips_pv[25])
                if tb == 18:
                    pending_pv.extend(strips_pv[26] + strips_pv[27])
                if tb == 19:
                    for stb in range(28, 32):
                        pending_pv.extend(strips_pv[stb])
            flush_pv()
            while epi_q:
                emit_epi_j(epi_q.pop(0))
            emit_epi_batch([5, 6, 7], on_act=False)
            emit_out_dma("lo")
            emit_epi_batch([8, 9, 10, 11], on_act=True)
            emit_out_dma("q8_11")
            emit_epi_batch([12, 13, 14, 15], on_act=False)
            emit_out_dma("q12_15")

    nc.compile()
    return nc


def _get_program():
    if "nc" not in _prog_cache:
        _prog_cache["nc"] = _build_program()
    return _prog_cache["nc"]


def _host_blobs(Wq, bq, Wk, bk, Wv, bv):
    WqT = np.asarray(Wq).T.astype(np.float32)  # [C, 64]
    WkT = np.asarray(Wk).T.astype(np.float32)
    WvT = np.asarray(Wv).T.astype(np.float32)
    wv_pack = np.empty((128, 8 * 64), dtype=np.float32)
    for c in range(8):
        wv_pack[:, c * 64:(c + 1) * 64] = WvT[c * 128:(c + 1) * 128, :]
    def w8(WT):
        o = np.empty((128, 4 * 128), dtype=np.float32)
        for gp in range(4):
            o[:, gp * 128:gp * 128 + 64] = WT[2 * gp * 128:(2 * gp + 1) * 128, :]
            o[:, gp * 128 + 64:(gp + 1) * 128] = WT[(2 * gp + 1) * 128:(2 * gp + 2) * 128, :]
        return o.astype(FP8NP)
    bv_rep = np.ascontiguousarray(np.broadcast_to(
        np.asarray(bv).astype(np.float32)[None, :], (128, HS)).astype(BF16))
    masks = _host_masks()
    b2m = (_NMASK + 1) * 256
    blob1 = np.zeros((128, 1032), dtype=np.uint8)
    blob1[:, 0:512] = w8(WkT).view(np.uint8)
    blob1[:, 512:1024] = w8(WqT).view(np.uint8)
    blob1[0:64, 1024:1028] = np.asarray(bk).astype(np.float32).reshape(
        HS, 1).view(np.uint8)
    blob1[0:64, 1028:1032] = np.asarray(bq).astype(np.float32).reshape(
        HS, 1).view(np.uint8)
    blob2 = np.zeros((128, b2m + 1152), dtype=np.uint8)
    blob2[:, 0:b2m] = masks.view(np.uint8)
    blob2[:, b2m:b2m + 1024] = wv_pack.astype(BF16).view(np.uint8)
    blob2[:, b2m + 1024:b2m + 1152] = bv_rep.view(np.uint8)
    return blob1, blob2


def _host_x8(xrows):
    """[N, C] rows -> fp8 [128, (N//512)*4096], 512-row groups of
    [gp(4)][two(2)][512] c-pair k-outer layout."""
    n = xrows.shape[0]
    ng = n // 512
    o = np.empty((128, ng * 4096), dtype=np.float32)
    xT = xrows.T  # [C, N]
    for g in range(ng):
        for gp in range(4):
            base = g * 4096 + gp * 1024
            o[:, base:base + 512] = xT[2 * gp * 128:(2 * gp + 1) * 128,
                                       g * 512:(g + 1) * 512]
            o[:, base + 512:base + 1024] = xT[(2 * gp + 1) * 128:(2 * gp + 2) * 128,
                                              g * 512:(g + 1) * 512]
    return np.ascontiguousarray(o.astype(FP8NP))


def _host_xt(xb):
    """[T, C] -> bf16 [128, 8*4096], half-range-major: partition p holds,
    per half-range hr, 8 chunks of 512 t columns of x^T row c*128+p."""
    xT = np.asarray(xb).T.astype(BF16).reshape(8, 128, T)  # [c, p, t]
    o = np.empty((128, 8 * 4096), dtype=BF16)
    for hr, (lo, hi) in enumerate(_HR):
        blk = np.transpose(xT[:, :, lo:hi], (1, 0, 2))  # [p, c, 512]
        o[:, hr * 4096:(hr + 1) * 4096] = blk.reshape(128, 4096)
    return np.ascontiguousarray(o)


def kernel(x, Wq, bq, Wk, bk, Wv, bv):
    from concourse.bass_utils import run_bass_kernel_spmd

    x = np.asarray(x, dtype=np.float32)
    blob1, blob2 = _host_blobs(Wq, bq, Wk, bk, Wv, bv)

    nc = _get_program()
    in_maps = []
    for b in range(NCORES):
        xb = x[b]
        in_maps.append({
            "xt": _host_xt(xb),
            "xt8": _host_x8(xb),
            "xq8": _host_x8(xb[KEEP[:1024]]),
            "blob1": blob1,
            "blob2": blob2,
        })
    res = run_bass_kernel_spmd(nc, in_maps, core_ids=list(range(NCORES)),
                               trace=TRACE, **TRACE_KW)
    global LAST_RESULTS
    LAST_RESULTS = res
    out = np.stack([res.results[b]["out"] for b in range(NCORES)], axis=0)
    return out.astype(np.float32)


# revision 62
# speedup vs baseline: 1.0256x; 1.0256x over previous
"""Sparse-attention head kernel for Trainium2, data-parallel over batch on 8 cores.

v6 design (per core, one batch):
  - DMA: all large input transfers ride the sync HW queue as few, large
    dma_starts with 4-8KB contiguous runs per partition (host lays x out
    half-range-major).  Consts ride the gpsimd queue.  The fp8 copy of the
    gathered-q tail rows is NOT transferred: keep[1024:2048] == t 3072..4095,
    so q-chunks 2,3 reuse xt8 groups 6,7.
  - q and k are projected W-stationary with fp8 DoubleRow into transposed
    fp8 layouts (qgT [64, 2*2048], kT [64, 1024/group]); v is projected
    x-stationary bf16 per t-block, with the bias-add on the Pool engine.
  - S runs fp8 DoubleRow (zero-padded pair slot): 2x column rate.
  - exp on ACT with the 1/sqrt(C) scale folded; boundary-block masks
    multiply e on DVE.
  - PV accumulates out^T into two pinned PSUM tiles [65, 1024] (row 64 =
    softmax denominator via the ones column); first writer per PSUM column
    region uses start=True (no explicit zeroing pass).
  - epilogue: j0..3 staged during the sweep; j4..7 and j8..15 batched
    (single PSUM->SBUF copy, packed PE transposes, batched reciprocal,
    scales split across ACT and DVE).
"""

import math
import os

if "JAX_PLATFORMS" not in os.environ:
    os.environ["JAX_PLATFORMS"] = "axon,cpu"

import numpy as np
import ml_dtypes

B, T, C = 8, 4096, 1024
HS = 64
KQ = T // 2
NCORES = 8
SCALE = float(C) ** -0.5
BF16 = ml_dtypes.bfloat16
FP8NP = ml_dtypes.float8_e4m3

_NT = T // 128   # 32
_NJ = KQ // 128  # 16
_NG = T // 512   # 8 k-projection groups
_NQ = KQ // 512  # 4 q-projection chunks (2,3 alias xt8 g6,g7)

# half-ranges of t, in DMA/layout order (tail first: prologue needs it)
_HR = [(3072, 3584), (3584, 4096), (0, 512), (512, 1024), (1024, 1536),
       (1536, 2048), (2048, 2560), (2560, 3072)]
_HRIDX = {lo // 512: i for i, (lo, hi) in enumerate(_HR)}


def _keep_indices(t):
    a = math.ceil(t / 4)
    keep = [t - 1 - x for x in range(a)]
    keep += [t - 1 - math.ceil(3 / a * (x - a) ** 2 + a) for x in range(a, math.ceil(t / 2))]
    return np.array(list(reversed(keep)), dtype=np.int64)


KEEP = _keep_indices(T)

_FULL, _BOUND, _DEAD = 0, 1, 2
_BLOCK_KIND = np.empty((_NT, _NJ), dtype=np.int64)
for _tb in range(_NT):
    for _j in range(_NJ):
        qlo, qhi = KEEP[_j * 128], KEEP[_j * 128 + 127]
        if 128 * _tb + 127 <= qlo:
            _BLOCK_KIND[_tb, _j] = _FULL
        elif 128 * _tb > qhi:
            _BLOCK_KIND[_tb, _j] = _DEAD
        else:
            _BLOCK_KIND[_tb, _j] = _BOUND

_MASK_IDX = {}
for _tb in range(_NT):
    for _j in range(8):
        if _BLOCK_KIND[_tb, _j] == _BOUND:
            _MASK_IDX[(_tb, _j)] = len(_MASK_IDX)
_NMASK = len(_MASK_IDX)
_TRIL_IDX = _NMASK

_JLO = np.empty(_NT, dtype=np.int64)
for _tb in range(_NT):
    _JLO[_tb] = _NJ
    for _j in range(_NJ):
        if _BLOCK_KIND[_tb, _j] != _DEAD:
            _JLO[_tb] = _j
            break

# last alive t-block per q-block j (column j's accumulation completes there)
_JLAST = np.empty(_NJ, dtype=np.int64)
for _j in range(_NJ):
    _JLAST[_j] = (int(KEEP[_j * 128 + 127]) // 128)


def _host_masks():
    m = np.zeros((128, (_NMASK + 1) * 128), dtype=np.float32)
    for (tb, j), idx in _MASK_IDX.items():
        tvals = 128 * tb + np.arange(128)[:, None]
        kvals = KEEP[j * 128:(j + 1) * 128][None, :]
        m[:, idx * 128:(idx + 1) * 128] = (tvals <= kvals).astype(np.float32)
    r = np.arange(128)[:, None]
    c = np.arange(128)[None, :]
    m[:, _TRIL_IDX * 128:(_TRIL_IDX + 1) * 128] = (r <= c).astype(np.float32)
    return m.astype(BF16)


_prog_cache = {}
TRACE = False
TRACE_KW = {}
LAST_RESULTS = None


_KNOBS = {
    "WARMUP_N": 4,     # PE warm-up dummy matmuls
    "PV_KEEP": 6,      # pending PV depth
    "EP_BUFS": 8,      # e-tile pool depth
    "MASK_POOL": 0,    # mask multiplies on Pool instead of DVE
}


def _knob(name):
    return int(os.environ.get(f"K_{name}", _KNOBS[name]))


def _build_program():
    import concourse.bass as bass  # noqa: F401
    import concourse.mybir as mybir
    import concourse.tile as tile
    from concourse import bacc
    from concourse.masks import make_identity

    dt = mybir.dt
    f32, bf16, fp8 = dt.float32, dt.bfloat16, dt.float8e4
    Alu = mybir.AluOpType
    Act = mybir.ActivationFunctionType
    DR = mybir.MatmulPerfMode.DoubleRow

    nc = bacc.Bacc("TRN2", target_bir_lowering=False, debug=False,
                   enable_partition_id=False)

    # x bf16, half-range-major: [128, hr(8) x c(8) x 512]
    xt_d = nc.dram_tensor("xt", [128, 8 * 4096], bf16, kind="ExternalInput").ap()
    # fp8 c-pair k-outer: per group g (512 t): 4096 cols = [gp(4)][two(2)][512]
    xt8_d = nc.dram_tensor("xt8", [128, _NG * 4096], fp8, kind="ExternalInput").ap()
    # only q-chunks 0,1 (quadratic keep rows); chunks 2,3 alias xt8 g6,g7
    xq8_d = nc.dram_tensor("xq8", [128, 2 * 4096], fp8, kind="ExternalInput").ap()
    # consts packed into two byte blobs (one dma_start each: per-dma
    # sequencer cost on the issuing engine is ~1us)
    # blob1: wk8 (512B) | wq8 (512B) | bk f32 (4B, parts 0..63) | bq f32
    blob1_d = nc.dram_tensor("blob1", [128, 1032], dt.uint8,
                             kind="ExternalInput").ap()
    # blob2: masks ((_NMASK+1)*256 B) | wv (1024B) | bv (128B)
    _B2M = (_NMASK + 1) * 256
    blob2_d = nc.dram_tensor("blob2", [128, _B2M + 1152], dt.uint8,
                             kind="ExternalInput").ap()
    out_d = nc.dram_tensor("out", [KQ, HS], f32, kind="ExternalOutput").ap()

    with tile.TileContext(nc) as tc:
        with (
            tc.tile_pool(name="const", bufs=1) as constp,
            tc.tile_pool(name="xt", bufs=1) as xtp,
            tc.tile_pool(name="proj", bufs=1) as projp,
            tc.tile_pool(name="psS", bufs=2, space="PSUM") as psS,
            tc.tile_pool(name="psO", bufs=1, space="PSUM") as psO,
            tc.tile_pool(name="work", bufs=2) as workp,
            tc.tile_pool(name="ework", bufs=_knob("EP_BUFS")) as ep,
        ):
            # ---- consts: two blob dma_starts on the scalar HW queue (fast
            # start; per-dma sequencer cost is ~1us, so batch them).  xt8
            # g7 rides between them so it transfers in parallel with g6 on
            # the sync queue. ----
            blob1 = constp.tile([128, 1032], dt.uint8)
            nc.scalar.dma_start(out=blob1, in_=blob1_d)
            blob2 = constp.tile([128, _B2M + 1152], dt.uint8)
            nc.scalar.dma_start(out=blob2, in_=blob2_d)
            wk8_sb = blob1[:, 0:512].bitcast(fp8)
            wq8_sb = blob1[:, 512:1024].bitcast(fp8)
            bk_sb = blob1[0:64, 1024:1028].bitcast(f32)
            bq_sb = blob1[0:64, 1028:1032].bitcast(f32)
            mask_big = blob2[:, 0:_B2M].bitcast(bf16)
            wv_sb = blob2[:, _B2M:_B2M + 1024].bitcast(bf16)
            bv_bc = blob2[:, _B2M + 1024:_B2M + 1152].bitcast(bf16)
            # PE warm-up: dummy matmuls on a zeroed tile while input DMAs
            # are in flight, so the PE p-state is ramped when real work
            # lands (the dummy memset is first in the gpsimd stream).
            dummy = constp.tile([128, 512], bf16)
            nc.gpsimd.memset(dummy, 0.0)
            for _w in range(_knob("WARMUP_N")):
                ps_w = psS.tile([128, 512], f32, name="ps", tag="ps")
                nc.tensor.matmul(ps_w, lhsT=dummy[:, 0:128], rhs=dummy,
                                 start=True, stop=True)

            # ---- persistent tensors ----
            xt_h = [xtp.tile([128, 4096], bf16, name=f"xth_{i}",
                             tag=f"xth_{i}") for i in range(8)]
            xt8_g = [xtp.tile([128, 4096], fp8, name=f"xt8_{g}",
                              tag=f"xt8_{g}") for g in range(_NG)]
            xq8_g = [xtp.tile([128, 4096], fp8, name=f"xq8_{g}",
                              tag=f"xq8_{g}") for g in range(2)]
            # S-critical zero halves go on DVE (idle until ~6us; Pool is
            # busy with the other memsets); identity last (epilogue-only)
            kt_g = [projp.tile([64, 1024], fp8, name=f"kt_{g}",
                               tag=f"kt_{g}") for g in range(_NG)]
            qgt8 = projp.tile([64, 2 * KQ], fp8)
            nc.vector.memset(qgt8[:, KQ:2 * KQ].bitcast(dt.uint32), 0)
            nc.vector.memset(kt_g[6][:, 512:1024].bitcast(dt.uint32), 0)
            nc.vector.memset(kt_g[7][:, 512:1024].bitcast(dt.uint32), 0)
            for g in (0, 1, 2, 3, 4, 5):
                nc.gpsimd.memset(kt_g[g][:, 512:1024].bitcast(dt.uint32), 0)
            vext_sb = [projp.tile([128, HS + 1], bf16, name=f"vext_{tb}",
                                  tag=f"vext_{tb}") for tb in range(_NT)]
            for tb in range(_NT):
                nc.gpsimd.memset(vext_sb[tb][:, HS:HS + 1], 1.0)
            ident_f = constp.tile([128, 128], f32)
            make_identity(nc, ident_f)
            ps_o_lo = psO.tile([65, 1024], f32, name="po_lo", tag="po_lo")
            ps_o_hi = psO.tile([65, 1024], f32, name="po_hi", tag="po_hi")

            # ---- big input DMAs, all on the sync HW queue, in need-order ----
            def dma_xt8(g):
                nc.sync.dma_start(out=xt8_g[g],
                                  in_=xt8_d[:, g * 4096:(g + 1) * 4096])

            def dma_xth(i):
                nc.sync.dma_start(out=xt_h[i],
                                  in_=xt_d[:, i * 4096:(i + 1) * 4096])

            def dma_xq8(c):
                nc.sync.dma_start(out=xq8_g[c],
                                  in_=xq8_d[:, c * 4096:(c + 1) * 4096])

            # order feeds the ACT exp stream continuously: k-data first
            # (unlocks strip + hi-prepass S/exp with PV deferred), then the
            # quadratic-q and v data for the dense sweep, with the strips' v
            # ranges (t 3072..4095) arriving late since their PV is deferred.
            dma_xt8(6)
            dma_xt8(7)
            dma_xt8(0)
            dma_xt8(1)
            dma_xq8(0)
            dma_xq8(1)
            dma_xth(2)   # t 0..511    (v tb0..3)
            dma_xth(3)   # t 512..1023
            dma_xt8(2)
            dma_xth(4)   # t 1024..1535
            dma_xt8(3)
            dma_xth(5)
            dma_xt8(4)
            dma_xth(6)   # t 2048..2559
            dma_xt8(5)
            dma_xth(7)   # t 2560..3071 (v tb20..23)
            dma_xth(0)   # t 3072..3583 (v tb24..27, strips)
            dma_xth(1)   # t 3584..4095 (v tb28..31, strips)

            def xt_sl(c, lo, hi):
                hr = _HRIDX[lo // 512]
                o = c * 512 + (lo - _HR[hr][0])
                return xt_h[hr][:, o:o + (hi - lo)]

            # ---- W-stationary fp8-DR projections (k and q) ----
            def emit_kproj(g):
                ps_k = psS.tile([64, 512], f32, name="ps", tag="ps")
                for gp in range(4):
                    lhs = wk8_sb[:, gp * 128:(gp + 1) * 128].rearrange(
                        "p (two m) -> p two m", two=2)
                    rhs = xt8_g[g][:, gp * 1024:(gp + 1) * 1024].rearrange(
                        "p (two n) -> p two n", two=2)
                    nc.tensor.matmul(ps_k, lhsT=lhs, rhs=rhs,
                                     start=(gp == 0), stop=(gp == 3),
                                     perf_mode=DR)
                nc.vector.tensor_scalar(out=kt_g[g][:, 0:512], in0=ps_k,
                                        scalar1=bk_sb, scalar2=None,
                                        op0=Alu.add)

            def emit_qproj(c):
                src = xq8_g[c] if c < 2 else xt8_g[4 + c]
                ps_q = psS.tile([64, 512], f32, name="ps", tag="ps")
                for gp in range(4):
                    lhs = wq8_sb[:, gp * 128:(gp + 1) * 128].rearrange(
                        "p (two m) -> p two m", two=2)
                    rhs = src[:, gp * 1024:(gp + 1) * 1024].rearrange(
                        "p (two n) -> p two n", two=2)
                    nc.tensor.matmul(ps_q, lhsT=lhs, rhs=rhs,
                                     start=(gp == 0), stop=(gp == 3),
                                     perf_mode=DR)
                nc.vector.tensor_scalar(out=qgt8[:, c * 512:(c + 1) * 512],
                                        in0=ps_q, scalar1=bq_sb, scalar2=None,
                                        op0=Alu.add)

            # ---- v projection per t-block (x-stationary; the 8-matmul
            # accumulation burst pipelines at stream rate).  When a host
            # PSUM slice is provided (unused tail of an S tile), no psS pool
            # allocation happens — keeping the pool rotation at 2 allocs per
            # t-block avoids a full-exp-duration WAR bubble. ----
            def emit_vproj(tb, ps_slice=None):
                t0 = tb * 128
                ps_v = ps_slice
                skip = ps_slice is not None
                if ps_v is None:
                    ps_v = psS.tile([128, 64], f32, name="ps", tag="ps")
                for c in range(8):
                    nc.tensor.matmul(ps_v, lhsT=xt_sl(c, t0, t0 + 128),
                                     rhs=wv_sb[:, c * 64:(c + 1) * 64],
                                     start=(c == 0), stop=(c == 7),
                                     skip_group_check=skip)
                nc.vector.tensor_tensor(out=vext_sb[tb][:, 0:HS], in0=ps_v,
                                        in1=bv_bc, op=Alu.add)

            # ---- attention ----
            def s_matmul(ps_slice, tb, q0, q1):
                g, o = tb // 4, (tb % 4) * 128
                lhs = kt_g[g].rearrange("p (two m) -> p two m",
                                        two=2)[:, :, o:o + 128]
                rhs = qgt8.rearrange("p (two n) -> p two n", two=2)[:, :, q0:q1]
                nc.tensor.matmul(ps_slice, lhsT=lhs, rhs=rhs,
                                 start=True, stop=True, perf_mode=DR)

            pending_pv = []

            def flush_pv(keep=0):
                while len(pending_pv) > keep:
                    tb, e_sl, q0, qm, stop = pending_pv.pop(0)
                    p0 = q0
                    while p0 < qm:
                        pb = min((p0 // 512 + 1) * 512, qm)
                        tgt = ps_o_lo if p0 < 1024 else ps_o_hi
                        base = 0 if p0 < 1024 else 1024
                        # first writer zeroes its own columns (no init pass):
                        # tb0 writes all lo columns via att_lo(0) and all hi
                        # columns via the deferred hi-prepass entry.
                        nc.tensor.matmul(
                            tgt[:, p0 - base:pb - base], lhsT=vext_sb[tb],
                            rhs=e_sl[:, p0 - q0:pb - q0],
                            start=(tb == 0), stop=stop, skip_group_check=True)
                        p0 = pb

            def mask_block(e_sb, tb, j, o):
                midx = None
                if j < 8 and _BLOCK_KIND[tb, j] == _BOUND:
                    midx = _MASK_IDX[(tb, j)]
                elif j >= 8 and tb == j + 16:
                    midx = _TRIL_IDX
                if midx is not None:
                    eng = nc.gpsimd if _knob("MASK_POOL") else nc.vector
                    eng.tensor_tensor(
                        out=e_sb[:, o:o + 128], in0=e_sb[:, o:o + 128],
                        in1=mask_big[:, midx * 128:(midx + 1) * 128],
                        op=Alu.mult)

            def emit_att_lo_pair(tbA, tbB, fillers=()):
                # merged lo chunk for two t-blocks (their lo spans fit one
                # PSUM tile together): one exp instead of two, fewer
                # pipeline transitions
                qA, qB = int(_JLO[tbA]) * 128, int(_JLO[tbB]) * 128
                wA, wB = 1024 - qA, 1024 - qB
                assert wA + wB <= 1024
                ps_s = psS.tile([128, 1024], f32, name="ps", tag="ps")
                for s0 in range(qA, 1024, 512):
                    s1 = min(s0 + 512, 1024)
                    s_matmul(ps_s[:, s0 - qA:s1 - qA], tbA, s0, s1)
                for s0 in range(qB, 1024, 512):
                    s1 = min(s0 + 512, 1024)
                    s_matmul(ps_s[:, wA + s0 - qB:wA + s1 - qB], tbB, s0, s1)
                prev = list(pending_pv)
                pending_pv.clear()
                e_sb = ep.tile([128, 1024], bf16, name="e_sb")
                nc.scalar.activation(e_sb[:, 0:wA + wB], ps_s[:, 0:wA + wB],
                                     Act.Exp, scale=SCALE)
                for j in range(qA // 128, 8):
                    mask_block(e_sb, tbA, j, j * 128 - qA)
                for j in range(qB // 128, 8):
                    mask_block(e_sb, tbB, j, wA + j * 128 - qB)
                pending_pv.extend(prev)
                flush_pv(keep=_knob("PV_KEEP"))
                pending_pv.append((tbA, e_sb[:, 0:wA], qA, 1024, False))
                pending_pv.append((tbB, e_sb[:, wA:wA + wB], qB, 1024, False))
                ctx = {"ps": ps_s, "next": 1024, "limit": wA + wB}
                for f in fillers:
                    f(ctx)

            def emit_att(tb, q0, q1, stop=False, fillers=()):
                first = True
                while q0 < q1:
                    qm = min((q0 // 1024 + 1) * 1024, q1)
                    ps_s = psS.tile([128, 1024], f32, name="ps", tag="ps")
                    for s0 in range(q0, qm, 512):
                        s1 = min(s0 + 512, qm)
                        s_matmul(ps_s[:, s0 - q0:s1 - q0], tb, s0, s1)
                    prev = list(pending_pv)
                    pending_pv.clear()
                    e_sb = ep.tile([128, 1024], bf16, name="e_sb")
                    w = qm - q0
                    nc.scalar.activation(e_sb[:, 0:w], ps_s[:, 0:w],
                                         Act.Exp, scale=SCALE)
                    for j in range(q0 // 128, qm // 128):
                        mask_block(e_sb, tb, j, j * 128 - q0)
                    pending_pv.extend(prev)
                    flush_pv(keep=_knob("PV_KEEP"))
                    pending_pv.append((tb, e_sb[:, 0:w], q0, qm, stop))
                    if first:
                        # off-critical-path work lands between the S chunks;
                        # fillers may claim unused 64-col tails of this
                        # chunk's PSUM tile (avoids extra pool allocations)
                        ctx = {"ps": ps_s, "next": 1024, "limit": w}
                        for f in fillers:
                            f(ctx)
                        first = False
                    q0 = qm

            def _ps64(ctx):
                # carve a [128, 64] slice off the current S tile's free tail
                if ctx is not None and ctx["next"] - 64 >= ctx["limit"]:
                    ctx["next"] -= 64
                    return ctx["ps"][:, ctx["next"]:ctx["next"] + 64]
                return None

            # ---- deferred S/exp producers (PV injected later) ----
            # strips (t-blocks 24..31) and the hi-prepass (tb 0..5 over
            # q 1024..2048) run S+exp early, holding e in dedicated tiles;
            # their PV entries are injected into pending_pv once the
            # corresponding vext data has arrived.
            estrip = [projp.tile([128, 1024], bf16, name=f"es_{i}",
                                 tag=f"es_{i}") for i in range(6)]
            eahi = [projp.tile([128, 1024], bf16, name=f"eh_{tb}",
                               tag=f"eh_{tb}") for tb in range(8)]
            strips_pv = {tb: [] for tb in range(24, 32)}
            ahi_pv = {}
            _strip_tile = [0]

            def emit_strip_pair(p):
                j0 = 8 + 2 * p
                items = []
                for tb in range(24, 26 + 2 * p):
                    c0 = max(j0 * 128, (tb - 16) * 128)
                    c1 = (j0 + 2) * 128
                    if c0 < c1:
                        items.append((tb, c0, c1))

                def flush_group(g):
                    if not g:
                        return
                    ps_s = psS.tile([128, 1024], f32, name="ps", tag="ps")
                    ofs = 0
                    placed = []
                    for (tb, c0, c1) in g:
                        s_matmul(ps_s[:, ofs:ofs + (c1 - c0)], tb, c0, c1)
                        placed.append((tb, c0, c1, ofs))
                        ofs += c1 - c0
                    e_sb = estrip[_strip_tile[0]]
                    _strip_tile[0] += 1
                    nc.scalar.activation(e_sb[:, 0:ofs], ps_s[:, 0:ofs],
                                         Act.Exp, scale=SCALE)
                    for (tb, c0, c1, o) in placed:
                        for j in range(c0 // 128, c1 // 128):
                            mask_block(e_sb, tb, j, o + j * 128 - c0)
                    for (tb, c0, c1, o) in placed:
                        strips_pv[tb].append(
                            (tb, e_sb[:, o:o + (c1 - c0)], c0, c1, False))

                g, used = [], 0
                for it in items:
                    wdt = it[2] - it[1]
                    if used + wdt > 1024:
                        flush_group(g)
                        g, used = [], 0
                    g.append(it)
                    used += wdt
                flush_group(g)

            def emit_att_hi(tb):
                # hi-prepass: S+exp for q 1024..2048 of t-block tb, deferred
                ps_s = psS.tile([128, 1024], f32, name="ps", tag="ps")
                s_matmul(ps_s[:, 0:512], tb, 1024, 1536)
                s_matmul(ps_s[:, 512:1024], tb, 1536, 2048)
                e_sb = eahi[tb]
                nc.scalar.activation(e_sb, ps_s, Act.Exp, scale=SCALE)
                ahi_pv[tb] = (tb, e_sb[:, 0:1024], 1024, 2048, False)

            # ---- epilogue ----
            out8_lo = workp.tile([128, 8 * HS], f32, name="out8lo", tag="o8lo")
            out8_hi = workp.tile([128, 8 * HS], f32, name="out8hi", tag="o8hi")

            def emit_epi_j(j):
                # staged single-j epilogue (used for early-finishing lo j's)
                ps_o = ps_o_lo if j < 8 else ps_o_hi
                base = 0 if j < 8 else 1024
                out8 = out8_lo if j < 8 else out8_hi
                jj = j % 8
                ot = workp.tile([65, 128], f32, name="ot", tag=f"ot{j % 2}")
                nc.vector.tensor_copy(
                    ot, ps_o[:, j * 128 - base:(j + 1) * 128 - base])
                ps_on = psS.tile([128, 65], f32, name="ps", tag="ps")
                nc.tensor.transpose(ps_on, ot, ident_f[0:65, 0:65])
                rec = workp.tile([128, 1], f32, name="rec", tag=f"rec{j % 2}")
                nc.vector.reciprocal(rec, ps_on[:, HS:HS + 1])
                nc.vector.tensor_scalar(
                    out=out8[:, jj * HS:(jj + 1) * HS], in0=ps_on[:, 0:HS],
                    scalar1=rec[:, :1], scalar2=None, op0=Alu.mult)

            def emit_epi_batch(jlist, on_act):
                # batched: one PSUM->SBUF copy, packed transposes, one
                # reciprocal, per-j scale on ACT or DVE.
                n = len(jlist)
                j0 = jlist[0]
                ps_o = ps_o_lo if j0 < 8 else ps_o_hi
                base = 0 if j0 < 8 else 1024
                out8 = out8_lo if j0 < 8 else out8_hi
                ot = workp.tile([65, n * 128], f32, name=f"otb{j0}",
                                tag=f"otb{j0 % 2}")
                nc.vector.tensor_copy(
                    ot, ps_o[:, j0 * 128 - base:(j0 + n) * 128 - base])
                pack = psS.tile([128, 1024], f32, name="ps", tag="ps")
                for i in range(n):
                    nc.tensor.matmul(
                        pack[:, i * 65:(i + 1) * 65],
                        lhsT=ot[:, i * 128:(i + 1) * 128],
                        rhs=ident_f[0:65, 0:65], is_transpose=True,
                        skip_group_check=True)
                rec = workp.tile([128, n], f32, name=f"recb{j0}",
                                 tag=f"recb{j0 % 2}")
                packv = pack[:, 0:n * 65].rearrange("p (i c) -> p i c", c=65)
                nc.vector.reciprocal(rec, packv[:, :, HS:HS + 1])
                for i, j in enumerate(jlist):
                    jj = j % 8
                    if on_act:
                        nc.scalar.activation(
                            out8[:, jj * HS:(jj + 1) * HS],
                            pack[:, i * 65:i * 65 + HS],
                            Act.Copy, scale=rec[:, i:i + 1])
                    else:
                        nc.vector.tensor_scalar(
                            out=out8[:, jj * HS:(jj + 1) * HS],
                            in0=pack[:, i * 65:i * 65 + HS],
                            scalar1=rec[:, i:i + 1], scalar2=None,
                            op0=Alu.mult)

            def emit_out_dma(which):
                if which == "lo":
                    out8, qbase, j0, nj = out8_lo, 0, 0, 8
                elif which == "q8_11":
                    out8, qbase, j0, nj = out8_hi, 1024, 0, 4
                else:
                    out8, qbase, j0, nj = out8_hi, 1536, 4, 4
                out_view = out_d[qbase:qbase + nj * 128, :].rearrange(
                    "(j p) d -> p j d", p=128)
                nc.sync.dma_start(
                    out=out_view,
                    in_=out8[:, j0 * HS:(j0 + nj) * HS].rearrange(
                        "p (j d) -> p j d", j=nj))

            # ---- schedule ----
            # prologue ordered by data-readiness: strips 0-1 need only
            # g6 + q-chunk2, so they start as soon as the first k group
            # lands; the ACT exp stream then never starves.
            emit_kproj(6)
            emit_qproj(2)
            emit_strip_pair(0)
            emit_strip_pair(1)
            emit_kproj(7)
            emit_qproj(3)
            emit_kproj(0)
            emit_att_hi(0)
            emit_att_hi(1)
            emit_strip_pair(2)
            emit_strip_pair(3)
            emit_att_hi(2)
            emit_att_hi(3)
            emit_kproj(1)
            for tb in range(4, 8):
                emit_att_hi(tb)
            emit_qproj(0)
            emit_qproj(1)
            emit_vproj(0)
            emit_vproj(1)
            # dense sweep; v-proj leads by 2 t-blocks, k groups 2..5 and the
            # strip PV injections placed just behind their DMA arrivals.
            kg_at = {7: 2, 11: 3, 15: 4, 18: 5}
            epi_q = [0, 1, 2, 3, 4]  # staged lo epilogues by _JLAST
            for tb in range(0, 24):
                fillers = []
                if tb + 2 < 24:
                    fillers.append(lambda c, t=tb + 2: emit_vproj(t, _ps64(c)))
                if tb == 16 or tb == 17:
                    for stb in (24 + 2 * (tb - 16), 25 + 2 * (tb - 16)):
                        fillers.append(lambda c, t=stb: emit_vproj(t, _ps64(c)))
                if tb == 18:
                    for stb in range(28, 32):
                        fillers.append(lambda c, t=stb: emit_vproj(t, _ps64(c)))
                if tb in kg_at:
                    fillers.append(lambda c, g=kg_at[tb]: emit_kproj(g))
                # staging margin: pending PV holds ~PV_KEEP+1 entries
                # (~(PV_KEEP+1)/2 t-blocks), so column j's last PV piece is
                # only guaranteed emitted this many t-blocks after _JLAST
                margin = _knob("PV_KEEP") // 2 + 2
                while epi_q and _JLAST[epi_q[0]] <= tb - margin:
                    fillers.append(lambda c, j=epi_q.pop(0): emit_epi_j(j))
                if tb <= 7:
                    pending_pv.append(ahi_pv[tb])
                    emit_att(tb, int(_JLO[tb]) * 128, 1024, fillers=fillers)
                elif tb in (20, 22):
                    emit_att_lo_pair(tb, tb + 1, fillers=fillers)
                    emit_att(tb, 1024, 2048)
                elif tb in (21, 23):
                    emit_att(tb, 1024, 2048, stop=(tb == 23),
                             fillers=fillers)
                else:
                    emit_att(tb, int(_JLO[tb]) * 128, 2048, stop=False,
                             fillers=fillers)
                if tb == 17:
                    pending_pv.extend(strips_pv[24] + strips_pv[25])
                if tb == 18:
                    pending_pv.extend(strips_pv[26] + strips_pv[27])
                if tb == 19:
                    for stb in range(28, 32):
                        pending_pv.extend(strips_pv[stb])
            flush_pv()
            while epi_q:
                emit_epi_j(epi_q.pop(0))
            emit_epi_batch([5, 6, 7], on_act=False)
            emit_out_dma("lo")
            emit_epi_batch([8, 9, 10, 11], on_act=True)
            emit_out_dma("q8_11")
            emit_epi_batch([12, 13, 14, 15], on_act=False)
            emit_out_dma("q12_15")

    nc.compile()
    return nc


def _get_program():
    if "nc" not in _prog_cache:
        _prog_cache["nc"] = _build_program()
    return _prog_cache["nc"]


def _host_blobs(Wq, bq, Wk, bk, Wv, bv):
    WqT = np.asarray(Wq).T.astype(np.float32)  # [C, 64]
    WkT = np.asarray(Wk).T.astype(np.float32)
    WvT = np.asarray(Wv).T.astype(np.float32)
    wv_pack = np.empty((128, 8 * 64), dtype=np.float32)
    for c in range(8):
        wv_pack[:, c * 64:(c + 1) * 64] = WvT[c * 128:(c + 1) * 128, :]
    def w8(WT):
        o = np.empty((128, 4 * 128), dtype=np.float32)
        for gp in range(4):
            o[:, gp * 128:gp * 128 + 64] = WT[2 * gp * 128:(2 * gp + 1) * 128, :]
            o[:, gp * 128 + 64:(gp + 1) * 128] = WT[(2 * gp + 1) * 128:(2 * gp + 2) * 128, :]
        return o.astype(FP8NP)
    bv_rep = np.ascontiguousarray(np.broadcast_to(
        np.asarray(bv).astype(np.float32)[None, :], (128, HS)).astype(BF16))
    masks = _host_masks()
    b2m = (_NMASK + 1) * 256
    blob1 = np.zeros((128, 1032), dtype=np.uint8)
    blob1[:, 0:512] = w8(WkT).view(np.uint8)
    blob1[:, 512:1024] = w8(WqT).view(np.uint8)
    blob1[0:64, 1024:1028] = np.asarray(bk).astype(np.float32).reshape(
        HS, 1).view(np.uint8)
    blob1[0:64, 1028:1032] = np.asarray(bq).astype(np.float32).reshape(
        HS, 1).view(np.uint8)
    blob2 = np.zeros((128, b2m + 1152), dtype=np.uint8)
    blob2[:, 0:b2m] = masks.view(np.uint8)
    blob2[:, b2m:b2m + 1024] = wv_pack.astype(BF16).view(np.uint8)
    blob2[:, b2m + 1024:b2m + 1152] = bv_rep.view(np.uint8)
    return blob1, blob2


def _host_x8(xrows):
    """[N, C] rows -> fp8 [128, (N//512)*4096], 512-row groups of
    [gp(4)][two(2)][512] c-pair k-outer layout."""
    n = xrows.shape[0]
    ng = n // 512
    o = np.empty((128, ng * 4096), dtype=np.float32)
    xT = xrows.T  # [C, N]
    for g in range(ng):
        for gp in range(4):
            base = g * 4096 + gp * 1024
            o[:, base:base + 512] = xT[2 * gp * 128:(2 * gp + 1) * 128,
                                       g * 512:(g + 1) * 512]
            o[:, base + 512:base + 1024] = xT[(2 * gp + 1) * 128:(2 * gp + 2) * 128,
                                              g * 512:(g + 1) * 512]
    return np.ascontiguousarray(o.astype(FP8NP))


def _host_xt(xb):
    """[T, C] -> bf16 [128, 8*4096], half-range-major: partition p holds,
    per half-range hr, 8 chunks of 512 t columns of x^T row c*128+p."""
    xT = np.asarray(xb).T.astype(BF16).reshape(8, 128, T)  # [c, p, t]
    o = np.empty((128, 8 * 4096), dtype=BF16)
    for hr, (lo, hi) in enumerate(_HR):
        blk = np.transpose(xT[:, :, lo:hi], (1, 0, 2))  # [p, c, 512]
        o[:, hr * 4096:(hr + 1) * 4096] = blk.reshape(128, 4096)
    return np.ascontiguousarray(o)


def kernel(x, Wq, bq, Wk, bk, Wv, bv):
    from concourse.bass_utils import run_bass_kernel_spmd

    x = np.asarray(x, dtype=np.float32)
    blob1, blob2 = _host_blobs(Wq, bq, Wk, bk, Wv, bv)

    nc = _get_program()
    in_maps = []
    for b in range(NCORES):
        xb = x[b]
        in_maps.append({
            "xt": _host_xt(xb),
            "xt8": _host_x8(xb),
            "xq8": _host_x8(xb[KEEP[:1024]]),
            "blob1": blob1,
            "blob2": blob2,
        })
    res = run_bass_kernel_spmd(nc, in_maps, core_ids=list(range(NCORES)),
                               trace=TRACE, **TRACE_KW)
    global LAST_RESULTS
    LAST_RESULTS = res
    out = np.stack([res.results[b]["out"] for b in range(NCORES)], axis=0)
    return out.astype(np.float32)
